# revision 1
# baseline (speedup 1.0000x reference)
"""Multi-head causal attention (B=2, S=2048, H=1024, 16 heads x 64, RoPE) on 8 trn2 cores.

Sharding: tensor-parallel over heads (2 heads/core) for QKV+attention, then a
per-batch AllToAll switches to token-parallel for the output projection. Each
core owns 256 tokens of each batch; the host concatenates disjoint row slices.

Design (per core c, heads h0=2c, h0+1):
 - xT [1024, 4096] feature-major activations, host-transposed + bf16-cast, so
   both qT/kT (feature-major, via lhsT=W.T rhs=xT) and V (token-major, via a
   small PE transpose of vT) come straight out of matmuls - no activation
   transposes on device.
 - RoPE applied feature-major with host cos/sin tables; the half-rotation
   partition swap is an SBUF->SBUF DMA, sin-multiply on GpSimd, cos-multiply
   and add on DVE. qT/kT stored float32r (full PE rate at N>=512).
 - Scores computed TRANSPOSED: sT[k, q] = matmul(lhsT=kT_block, rhs=qT_chunk)
   with the two heads packed in row groups (0,0)/(64,0) -> softmax probs come
   out in the [k, q] layout PV wants; no probability transposes. Softmax max-
   subtraction is skipped (logits ~N(0,1); exp is safe in fp32). Causal mask =
   bf16 0/1 multiply on the diagonal block's probs.
 - exp on ACT writes bf16 probs; PV matmuls (lhsT=V[tok,d], rhs=probsT) pack
   the heads in col groups (0,0)/(0,64); per-q softmax sums accumulate in the
   same pass via 0/1 selector-column matmuls into PSUM rows 0/33.
 - Normalization: sums broadcast across partitions with a K=34 selector
   matmul, reciprocal on DVE, then fused into the PSUM->SBUF context copy.
 - Per-batch AllToAll on [8, 128, 256] bf16 ctxT blocks: batch 0'''s collective
   and out-proj overlap batch 1'''s attention. Received buffer is exactly
   ctx_shard.T = lhsT of the out-proj (x W_out.T, fp32 out).
"""

import numpy as np

import concourse.bacc as bacc
import concourse.mybir as mybir
import concourse.tile as tile
from concourse.bass_utils import run_bass_kernel_spmd

F32 = mybir.dt.float32
F32R = mybir.dt.float32r
BF16 = mybir.dt.bfloat16
EXP = mybir.ActivationFunctionType.Exp

B, S, H = 2, 2048, 1024
NH, HD = 16, 64
NCORES = 8
T = B * S            # 4096 flattened tokens (b-major)
TBLK = T // NCORES   # 512 tokens per a2a block
P = 128


def _build_nc():
    nc = bacc.Bacc(None, num_devices=NCORES)

    xT_d = nc.dram_tensor("xT", [H, T], BF16, kind="ExternalInput")
    wqkvT_d = nc.dram_tensor("wqkvT", [H, 384], BF16, kind="ExternalInput")
    woutT_d = nc.dram_tensor("woutT", [H, H], BF16, kind="ExternalInput")
    costab_d = nc.dram_tensor("costab", [P, S], F32, kind="ExternalInput")
    sintab_d = nc.dram_tensor("sintab", [P, S], F32, kind="ExternalInput")
    maskT_d = nc.dram_tensor("maskT", [P, P], BF16, kind="ExternalInput")
    identf_d = nc.dram_tensor("identf", [P, P], BF16, kind="ExternalInput")
    esel_d = nc.dram_tensor("esel", [P, 4], BF16, kind="ExternalInput")
    bsel_d = nc.dram_tensor("bsel", [34, P], F32, kind="ExternalInput")
    out_d = nc.dram_tensor("out", [TBLK, H], F32, kind="ExternalOutput")

    with tile.TileContext(nc) as tc:
        with (
            tc.tile_pool(name="long", bufs=1) as lp,
            tc.tile_pool(name="dram", bufs=1, space="DRAM") as dp,
        ):
            # long-lived tiles
            qT = [lp.tile([P, S], F32R, tag=f"qT{b}", name=f"qT{b}") for b in range(B)]
            kT = [lp.tile([P, S], F32R, tag=f"kT{b}", name=f"kT{b}") for b in range(B)]
            V = [[lp.tile([P, 16, HD], BF16, tag=f"V{b}{h}", name=f"V{b}{h}") for h in range(2)]
                 for b in range(B)]
            ctxT = [lp.tile([P, S], BF16, tag=f"ctxT{b}", name=f"ctxT{b}") for b in range(B)]
            maskT_t = lp.tile([P, P], BF16, tag="maskT")
            identf_t = lp.tile([P, P], BF16, tag="identf")
            esel_t = lp.tile([P, 4], BF16, tag="esel")
            bsel_t = lp.tile([34, P], F32R, tag="bsel")
            wo = lp.tile([P, 8, H], BF16, tag="wo")

            nc.sync.dma_start(maskT_t[:], maskT_d[:])
            nc.sync.dma_start(identf_t[:], identf_d[:])
            nc.sync.dma_start(esel_t[:], esel_d[:])
            nc.sync.dma_start(bsel_t[:], bsel_d[:].bitcast(F32R))

            a2a_in = [dp.tile([NCORES, P, 256], BF16, name=f"a2a_in{b}",
                              tag=f"a2a_in{b}") for b in range(B)]
            a2a_out = [dp.tile([NCORES, P, 256], BF16, name=f"a2a_out{b}",
                               tag=f"a2a_out{b}") for b in range(B)]

            # ---------------- Phase 1: QKV projection + RoPE + V transpose
            with (
                tc.tile_pool(name="p1c", bufs=1) as p1c,
                tc.tile_pool(name="p1s", bufs=3) as p1s,
                tc.tile_pool(name="p1v", bufs=2) as p1v,
                tc.tile_pool(name="p1t", bufs=4) as p1t,
                tc.tile_pool(name="ps1", bufs=6, space="PSUM") as ps1,
                tc.tile_pool(name="ps1t", bufs=2, space="PSUM") as ps1t,
            ):
                wq = p1c.tile([P, 8, 384], BF16, tag="wq")
                wqkv_r = wqkvT_d[:].rearrange("(k p) c -> p k c", p=P)
                nc.sync.dma_start(wq[:, 0:4, :], wqkv_r[:, 0:4, :])
                nc.sync.dma_start(wq[:, 4:8, :], wqkv_r[:, 4:8, :])
                costab_t = p1c.tile([P, S], F32, tag="costab")
                sintab_t = p1c.tile([P, S], F32, tag="sintab")

                # per 512-token chunk: 24 matmuls (q,k,v x 8 k-tiles)
                vts = {}  # pending v-chunk sbuf tiles for transposes
                for b in range(B):
                    for ch in range(4):
                        tok0 = b * S + ch * 512
                        c0 = ch * 512
                        xt_r = (xT_d[:, tok0:tok0 + 512]
                                .rearrange("(k p) t -> p k t", p=P))
                        xta = p1s.tile([P, 4, 512], BF16, tag="xta")
                        xtb = p1s.tile([P, 4, 512], BF16, tag="xtb")
                        nc.sync.dma_start(xta[:], xt_r[:, 0:4, :])
                        nc.sync.dma_start(xtb[:], xt_r[:, 4:8, :])
                        if b == 0 and ch == 0:
                            nc.sync.dma_start(costab_t[:], costab_d[:])
                            nc.sync.dma_start(sintab_t[:], sintab_d[:])
                        for m in range(3):
                            ps = ps1.tile([P, 512], F32, tag="qkv_ps")
                            for kt in range(8):
                                xt_half = xta if kt < 4 else xtb
                                nc.tensor.matmul(
                                    ps[:],
                                    wq[:, kt, m * P:(m + 1) * P],
                                    xt_half[:, kt % 4, :],
                                    start=(kt == 0), stop=(kt == 7),
                                )
                            if m < 2:
                                tgt = qT[b] if m == 0 else kT[b]
                                nc.vector.tensor_copy(tgt[:, c0:c0 + 512], ps[:])
                                swp = p1t.tile([P, 512], F32, tag="swp")
                                for g in range(4):
                                    dst = g * 32
                                    srcp = dst ^ 32
                                    nc.scalar.dma_start(
                                        swp[dst:dst + 32, :],
                                        tgt[srcp:srcp + 32, c0:c0 + 512]
                                        .bitcast(F32),
                                    )
                                nc.gpsimd.tensor_mul(
                                    swp[:], swp[:], sintab_t[:, c0:c0 + 512])
                                nc.vector.tensor_mul(
                                    tgt[:, c0:c0 + 512],
                                    tgt[:, c0:c0 + 512].bitcast(F32),
                                    costab_t[:, c0:c0 + 512])
                                nc.vector.tensor_add(
                                    tgt[:, c0:c0 + 512],
                                    tgt[:, c0:c0 + 512].bitcast(F32),
                                    swp[:])
                            else:
                                vt = p1v.tile([P, 512], BF16, tag="vT")
                                nc.scalar.copy(vt[:], ps[:])
                                vts[(b, ch)] = vt
                        # emit previous chunk's V transposes (software pipeline:
                        # keeps PE from stalling on the ACT copy)
                        for key in list(vts):
                            if key != (b, ch):
                                _v_transposes(nc, ps1t, V, identf_t, vts.pop(key), key)
                for key in list(vts):
                    _v_transposes(nc, ps1t, V, identf_t, vts.pop(key), key)

            nc.sync.dma_start(
                wo[:], woutT_d[:].rearrange("(j p) n -> p j n", p=P)
            )

            # ---------------- Phase 2: attention, transposed softmax
            with (
                tc.tile_pool(name="p2", bufs=4) as p2,
                tc.tile_pool(name="p2n", bufs=2) as p2n,
                tc.tile_pool(name="ps2s", bufs=4, space="PSUM") as ps2s,
                tc.tile_pool(name="ps2c", bufs=2, space="PSUM") as ps2c,
                tc.tile_pool(name="ps2m", bufs=2, space="PSUM") as ps2m,
                tc.tile_pool(name="ps2b", bufs=1, space="PSUM") as ps2b,
                tc.tile_pool(name="p3", bufs=2) as p3,
            ):
                ctxs0 = None
                sc_tiles = []
                for i_ in range(2):
                    sct = p2n.tile([34, 512], F32R, tag=f"sumrow{i_}",
                                   name=f"sc{i_}", bufs=1)
                    # rows 2:32 feed the K=34 broadcast matmul with zero
                    # weights -- zero once so they're finite (0 x NaN = NaN)
                    nc.vector.memset(sct[0:32, :].bitcast(F32), 0.0)
                    sc_tiles.append(sct)
                for b in range(B):
                    for qs in (3, 2, 1, 0):
                        pctx = ps2c.tile([P, 512], F32, tag="ctx")
                        psums = ps2m.tile([34, 512], F32, tag="sums",
                                          name="sums", bufs=1)
                        sc = sc_tiles[(b * 4 + qs) % 2]
                        nkb = 4 * qs + 4
                        pend = []  # pipelined PV work: (kb, h, probs, qoff, N)
                        for kb in range(nkb):
                            j = kb - 4 * qs
                            qoff = max(0, j) * P
                            N = 512 - qoff
                            for h in range(2):
                                psT = ps2s.tile([P, 512], F32, tag="sT")
                                nc.tensor.matmul(
                                    psT[:, 0:N],
                                    kT[b][h * HD:(h + 1) * HD, kb * P:(kb + 1) * P],
                                    qT[b][h * HD:(h + 1) * HD,
                                          qs * 512 + qoff:(qs + 1) * 512],
                                    start=True, stop=True,
                                    tile_position=(h * HD, 0),
                                )
                                pb = p2.tile([P, 512], BF16, tag="probs",
                                             bufs=10)
                                nc.scalar.activation(
                                    pb[:, 0:N], psT[:, 0:N], EXP, scale=0.125)
                                if j >= 0:
                                    # zero the strictly-upper triangle of the
                                    # diagonal 128-block (bf16 binary mask)
                                    nc.vector.tensor_mul(
                                        pb[:, 0:P], pb[:, 0:P], maskT_t[:])
                                pend.append((kb, h, pb, qoff, N))
                            # emit PV/sums one kb behind the score matmuls
                            while len(pend) > 8:
                                _pv_sums(nc, pctx, psums, V, esel_t, b, qs,
                                         nkb, pend.pop(0))
                        while pend:
                            _pv_sums(nc, pctx, psums, V, esel_t, b, qs, nkb,
                                     pend.pop(0))
                        # normalize: broadcast per-q sums, reciprocal, fused copy
                        nc.scalar.copy(sc[0:2, :], psums[0:2, :])
                        nc.scalar.copy(sc[32:34, :], psums[32:34, :])
                        # broadcast per-q sums across partitions: rows 0:64 get
                        # head0 sums, rows 64:128 head1 (K=34 0/1 selector,
                        # zero rows in the gap contribute nothing)
                        pbc = ps2b.tile([P, 512], F32, tag="bc")
                        nc.tensor.matmul(
                            pbc[:], bsel_t[0:34, :], sc[0:34, :],
                            start=True, stop=True)
                        rb = p2n.tile([P, 512], F32, tag="recip")
                        nc.vector.reciprocal(rb[:], pbc[:])
                        nc.vector.tensor_mul(
                            ctxT[b][:, qs * 512:(qs + 1) * 512], pctx[:], rb[:])
                        for half in range(2):
                            blk = 2 * qs + half
                            o0 = qs * 512 + half * 256
                            nc.scalar.dma_start(
                                a2a_in[b][blk], ctxT[b][:, o0:o0 + 256])
                    if b == 1:
                        # load + out-proj batch 0 BEFORE collective #1 so its
                        # PE/DMA work isn't queue-ordered behind it
                        ctxs0 = []
                        for jj in range(8):
                            cj = p3.tile([P, 256], BF16, tag=f"ctxs{jj}",
                                         name=f"ctxs0{jj}", bufs=2)
                            nc.sync.dma_start(cj[:], a2a_out[0][jj])
                            ctxs0.append(cj)
                        _out_proj(nc, tc, p3, ps2s, ctxs0, wo, out_d, 0)
                    # batch b's AllToAll overlaps batch b+1's attention
                    nc.gpsimd.collective_compute(
                        "AllToAll",
                        mybir.AluOpType.bypass,
                        replica_groups=[list(range(NCORES))],
                        ins=[a2a_in[b].opt()],
                        outs=[a2a_out[b].opt()],
                    )
                    if b == 0:
                        pass
                    else:
                        ctxs1 = []
                        for jj in range(8):
                            cj = p3.tile([P, 256], BF16, tag=f"ctxs{jj}",
                                         name=f"ctxs1{jj}", bufs=2)
                            nc.sync.dma_start(cj[:], a2a_out[1][jj])
                            ctxs1.append(cj)
                        _out_proj(nc, tc, p3, ps2s, ctxs1, wo, out_d, 1)

    nc.finalize()
    return nc


def _out_proj(nc, tc, p3, ps_pool, ctxs, wo, out_d, bb):
    for mt in range(2):
        for nt in range(2):
            po = ps_pool.tile([P, 512], F32, tag="sT", name="po")
            for jj in range(8):
                nc.tensor.matmul(
                    po[:],
                    ctxs[jj][:, mt * P:(mt + 1) * P],
                    wo[:, jj, nt * 512:(nt + 1) * 512],
                    start=(jj == 0), stop=(jj == 7),
                )
            ob = p3.tile([P, 512], F32, tag="ob", name="ob")
            nc.scalar.copy(ob[:], po[:])
            nc.scalar.dma_start(
                out_d[bb * 256 + mt * P:bb * 256 + (mt + 1) * P,
                      nt * 512:(nt + 1) * 512],
                ob[:])


def _v_transposes(nc, ps_pool, V, identf_t, vt, key):
    b, ch = key
    for h in range(2):
        for tb in range(4):
            pst = ps_pool.tile([P, HD], BF16, tag="vt_ps", name="vt_ps")
            nc.tensor.transpose(
                pst[:],
                vt[h * HD:(h + 1) * HD, tb * P:(tb + 1) * P],
                identf_t[h * HD:(h + 1) * HD, h * HD:(h + 1) * HD],
            )
            gb = ch * 4 + tb
            nc.vector.tensor_copy(V[b][h][:, gb, :], pst[:])


def _pv_sums(nc, pctx, psums, V, esel_t, b, qs, nkb, item):
    kb, h, pb, qoff, N = item
    nc.tensor.matmul(
        pctx[h * HD:(h + 1) * HD, qoff:512],
        V[b][h][:, kb, :],
        pb[:, 0:N],
        start=(kb == 0), stop=(kb == nkb - 1),
        tile_position=(0, h * HD),
    )
    nc.tensor.matmul(
        psums[h * 32:h * 32 + 2, qoff:512],
        esel_t[:, 2 * h:2 * h + 2],
        pb[:, 0:N],
        start=(kb == 0), stop=(kb == nkb - 1),
        tile_position=(0, h * 32),
        skip_group_check=True,
    )


_NC_CACHE = None


def _get_nc():
    global _NC_CACHE
    if _NC_CACHE is None:
        _NC_CACHE = _build_nc()
    return _NC_CACHE


def _host_tables():
    j = np.arange(32)
    inv = (10000.0 ** (-(j.astype(np.float64)) / 32.0))
    pos = np.arange(S, dtype=np.float64)
    fr = pos[:, None] * inv[None, :]              # [S, 32]
    cosT = np.cos(fr).T.astype(np.float32)        # [32, S]
    sinT = np.sin(fr).T.astype(np.float32)
    costab = np.tile(cosT, (4, 1))                # [128, S]
    sintab = np.concatenate([-sinT, sinT, -sinT, sinT], 0)
    import ml_dtypes
    kk = np.arange(P)[:, None]
    qq = np.arange(P)[None, :]
    maskT = np.where(kk <= qq, 1.0, 0.0).astype(ml_dtypes.bfloat16)
    identf = np.eye(P, dtype=np.float32).astype(ml_dtypes.bfloat16)
    return costab, sintab, maskT, identf


def _selectors():
    import ml_dtypes
    esel = np.zeros((P, 4), dtype=np.float32)
    esel[:, 0] = 1.0  # head0 sums -> psum row 0
    esel[:, 3] = 1.0  # head1 sums -> psum row 1
    esel = esel.astype(ml_dtypes.bfloat16)
    bsel = np.zeros((34, P), dtype=np.float32)
    bsel[0, 0:64] = 1.0    # head0 sums (psum row 0)
    bsel[33, 64:128] = 1.0  # head1 sums (psum row 33)
    return esel, bsel


def _make_in_maps(x, W_qkv, W_out):
    import ml_dtypes
    costab, sintab, maskT, identf = _host_tables()
    esel, bsel = _selectors()
    xT = np.ascontiguousarray(x.reshape(T, H).T).astype(ml_dtypes.bfloat16)
    woutT = np.ascontiguousarray(W_out.T).astype(ml_dtypes.bfloat16)
    in_maps = []
    for c in range(NCORES):
        h0 = 2 * c
        rows = np.concatenate([
            W_qkv[HD * h0:HD * (h0 + 2)],
            W_qkv[H + HD * h0:H + HD * (h0 + 2)],
            W_qkv[2 * H + HD * h0:2 * H + HD * (h0 + 2)],
        ], axis=0)                                        # [384, H]
        wqkvT = np.ascontiguousarray(rows.T).astype(ml_dtypes.bfloat16)
        in_maps.append({
            "xT": xT, "wqkvT": wqkvT, "woutT": woutT,
            "costab": costab, "sintab": sintab,
            "maskT": maskT, "identf": identf,
            "esel": esel, "bsel": bsel,
        })
    return in_maps


def _run_spmd(x, W_qkv, W_out, **kw):
    nc = _get_nc()
    in_maps = _make_in_maps(x, W_qkv, W_out)
    return run_bass_kernel_spmd(nc, in_maps, core_ids=list(range(NCORES)),
                                **kw)


def kernel(x, W_qkv, W_out):
    x = np.asarray(x, dtype=np.float32)
    W_qkv = np.asarray(W_qkv, dtype=np.float32)
    W_out = np.asarray(W_out, dtype=np.float32)
    res = _run_spmd(x, W_qkv, W_out)
    # core c owns tokens [c*256,(c+1)*256) of each batch (flattened b-major)
    full = np.empty((T, H), dtype=np.float32)
    for c in range(NCORES):
        o = res.results[c]["out"]
        full[c * 256:(c + 1) * 256] = o[0:256]
        full[S + c * 256:S + (c + 1) * 256] = o[256:512]
    return full.reshape(B, S, H)



# revision 6
# speedup vs baseline: 1.0651x; 1.0651x over previous
"""Multi-head causal attention (B=2, S=2048, H=1024, 16 heads x 64, RoPE) on 8 trn2 cores.

Sharding: tensor-parallel over heads (2 heads/core) for QKV+attention, then a
per-batch AllToAll switches to token-parallel for the output projection. Each
core owns 256 tokens of each batch; the host concatenates disjoint row slices.

Design (per core c, heads h0=2c, h0+1):
 - xT [1024, 4096] feature-major bf16 activations (host-transposed), so qT/kT
   (feature-major) come straight out of matmuls, and V comes out token-major
   by using the x-chunk as the stationary operand (no PE transposes at all).
 - RoPE: q/k weight ROWS are host-permuted to [0:16,32:48,16:32,48:64] per
   head so the rotate-half partner lives in the same 32-partition quadrant;
   the partition swap is then a single DVE stream_shuffle (no DMAs). cos
   multiply fused with the PSUM->SBUF copy on DVE, sin multiply on GpSimd,
   add on DVE. qT/kT stored bf16 (full PE rate at any N).
 - Scores TRANSPOSED: sT[k, q] = matmul(lhsT=kT_block, rhs=qT_chunk), both
   heads side by side in one [128, 1024] PSUM tile (2 banks) -> ONE exp
   instruction per key-block covers both heads (halves ACT instruction
   overhead). Max-subtraction skipped (logits ~N(0,1)). Causal mask = bf16
   0/1 multiply on the diagonal block's probs (both heads in one op).
 - V carries a ones-column (65-wide lhsT): the PV matmul emits the softmax
   denominators as PSUM row 64 for free - no separate selector matmuls.
 - Normalization: one DVE reciprocal (bf16 out) of the sums row, two K=1
   bf16 broadcast matmuls, two DVE multiplies fused into the PSUM->SBUF
   context copy.
 - Batch-1 QKV chunks are interleaved into batch-0's attention so the PE
   fills ACT-bound gaps and both phases overlap; batch 0's AllToAll and
   out-proj overlap batch 1's attention.
 - Per-batch AllToAll on [8, 128, 256] bf16 ctxT blocks issued from GpSimd
   (SWDGE) queues; received buffer is exactly ctx_shard.T = lhsT of the
   out-proj (x W_out.T, fp32 out).
"""

import numpy as np

import concourse.bacc as bacc
import concourse.mybir as mybir
import concourse.tile as tile
from concourse.bass_utils import run_bass_kernel_spmd

F32 = mybir.dt.float32
BF16 = mybir.dt.bfloat16
EXP = mybir.ActivationFunctionType.Exp

B, S, H = 2, 2048, 1024
NH, HD = 16, 64
NCORES = 8
T = B * S            # 4096 flattened tokens (b-major)
P = 128

# rotate-half partner lives 16 partitions away inside each 32-quadrant
SWAP_MASK = list(range(16, 32)) + list(range(0, 16))


def _build_nc():
    nc = bacc.Bacc(None, num_devices=NCORES)

    xT_d = nc.dram_tensor("xT", [H, T], BF16, kind="ExternalInput")
    wqkvT_d = nc.dram_tensor("wqkvT", [H, 384], BF16, kind="ExternalInput")
    woutT_d = nc.dram_tensor("woutT", [H, H], BF16, kind="ExternalInput")
    costab_d = nc.dram_tensor("costab", [P, S], F32, kind="ExternalInput")
    sintab_d = nc.dram_tensor("sintab", [P, S], F32, kind="ExternalInput")
    maskT2_d = nc.dram_tensor("maskT2", [P, 256], BF16, kind="ExternalInput")
    onesel_d = nc.dram_tensor("onesel", [P, 64], BF16, kind="ExternalInput")
    out_d = nc.dram_tensor("out", [2 * 256, H], F32, kind="ExternalOutput")

    with tile.TileContext(nc) as tc:
        with (
            tc.tile_pool(name="long", bufs=1) as lp,
            tc.tile_pool(name="dram", bufs=1, space="DRAM") as dp,
            tc.tile_pool(name="px", bufs=2) as px,
            tc.tile_pool(name="pswp", bufs=3) as pswp,
            tc.tile_pool(name="ppb", bufs=6) as ppb,
            tc.tile_pool(name="prb", bufs=2) as prb,
            tc.tile_pool(name="pob", bufs=2) as pob,
            tc.tile_pool(name="psA", bufs=2, space="PSUM") as psA,
            tc.tile_pool(name="psB", bufs=2, space="PSUM") as psB,
            tc.tile_pool(name="psC", bufs=1, space="PSUM") as psC,
        ):
            qT = [lp.tile([P, S], BF16, tag=f"qT{b}", name=f"qT{b}")
                  for b in range(B)]
            kT = [lp.tile([P, S], BF16, tag=f"kT{b}", name=f"kT{b}")
                  for b in range(B)]
            # V: per token-block gb, [h0 d0..63, ones | h1 d0..63, ones]
            V = [lp.tile([P, 16, 130], BF16, tag=f"V{b}", name=f"V{b}")
                 for b in range(B)]
            ctxT = [lp.tile([P, S], BF16, tag=f"ctxT{b}", name=f"ctxT{b}")
                    for b in range(B)]
            maskT2_t = lp.tile([P, 256], BF16, tag="maskT2")
            onesel_t = lp.tile([P, 64], BF16, tag="onesel")
            wq = lp.tile([P, 8, 384], BF16, tag="wq")
            wo = lp.tile([P, 8, H], BF16, tag="wo")
            costab_t = lp.tile([P, S], F32, tag="costab")
            sintab_t = lp.tile([P, S], F32, tag="sintab")

            a2a_in = [dp.tile([NCORES, P, 256], BF16, name=f"a2a_in{b}",
                              tag=f"a2a_in{b}") for b in range(B)]
            a2a_out = [dp.tile([NCORES, P, 256], BF16, name=f"a2a_out{b}",
                               tag=f"a2a_out{b}") for b in range(B)]

            wqkv_r = wqkvT_d[:].rearrange("(k p) c -> p k c", p=P)
            nc.sync.dma_start(wq[:, 0:4, :], wqkv_r[:, 0:4, :])
            nc.sync.dma_start(wq[:, 4:8, :], wqkv_r[:, 4:8, :])
            nc.sync.dma_start(costab_t[:], costab_d[:])
            nc.sync.dma_start(sintab_t[:], sintab_d[:])
            nc.sync.dma_start(maskT2_t[:], maskT2_d[:])
            nc.sync.dma_start(onesel_t[:], onesel_d[:])
            # ones columns of V (cols 64 and 129 of every token block)
            for b in range(B):
                vsel = V[b][:].rearrange("p g (s c) -> p g s c", s=2, c=65)
                nc.vector.memset(vsel[:, :, :, 64:65], 1.0)

            state = {"b1ch": 0}

            def chunk_qkv(b, ch):
                tok0 = b * S + ch * 512
                c0 = ch * 512
                xt_r = (xT_d[:, tok0:tok0 + 512]
                        .rearrange("(k p) t -> p k t", p=P))
                xta = px.tile([P, 4, 512], BF16, tag="xta")
                xtb = px.tile([P, 4, 512], BF16, tag="xtb")
                nc.sync.dma_start(xta[:], xt_r[:, 0:4, :])
                nc.sync.dma_start(xtb[:], xt_r[:, 4:8, :])
                w = slice(c0, c0 + 512)
                pss = []
                for m in range(2):      # q, k
                    ps = psA.tile([P, 512], F32, tag="qkv")
                    for kt in range(8):
                        xt_half = xta if kt < 4 else xtb
                        nc.tensor.matmul(
                            ps[:],
                            wq[:, kt, m * P:(m + 1) * P],
                            xt_half[:, kt % 4, :],
                            start=(kt == 0), stop=(kt == 7),
                        )
                    pss.append(ps)
                swps = []
                for m in range(2):      # rope: shuffle + cos (DVE), sin (Pool)
                    tgt = (qT[b] if m == 0 else kT[b])[:, w]
                    swp = pswp.tile([P, 512], F32, tag="swp")
                    swpb = pswp.tile([P, 512], BF16, tag="swpb")
                    nc.vector.stream_shuffle(swp[:], pss[m][:], SWAP_MASK)
                    nc.vector.tensor_mul(tgt, pss[m][:], costab_t[:, w])
                    nc.gpsimd.tensor_mul(swpb[:], swp[:], sintab_t[:, w])
                    swps.append(swpb)
                for m in range(2):
                    tgt = (qT[b] if m == 0 else kT[b])[:, w]
                    nc.vector.tensor_add(tgt, tgt, swps[m][:])
                for tb in range(4):     # v, token-major directly
                    ps = psA.tile([P, 512], F32, tag="qkv")
                    for kt in range(8):
                        xt_half = xta if kt < 4 else xtb
                        nc.tensor.matmul(
                            ps[:, 0:128],
                            xt_half[:, kt % 4, tb * P:(tb + 1) * P],
                            wq[:, kt, 256:384],
                            start=(kt == 0), stop=(kt == 7),
                        )
                    gb = ch * 4 + tb
                    vdst = (V[b][:, gb, 0:130]
                            .rearrange("p (s c) -> p s c", s=2, c=65))
                    vsrc = (ps[:, 0:128]
                            .rearrange("p (s c) -> p s c", s=2, c=64))
                    nc.vector.tensor_copy(vdst[:, :, 0:64], vsrc[:])

            def pv_emit(b, qs, nkb, pctx, item):
                kb, pb, qoff, N = item
                for h in range(2):
                    nc.tensor.matmul(
                        pctx[0:65, h * 512 + qoff:h * 512 + 512],
                        V[b][:, kb, 65 * h:65 * h + 65],
                        pb[:, h * 512 + qoff:h * 512 + 512],
                        start=(kb == 0), stop=(kb == nkb - 1),
                        skip_group_check=True,
                    )

            def att_block(b, qs):
                pctx = psC.tile([P, 1024], F32, tag="ctx")
                nkb = 4 * qs + 4
                pend = []
                for kb in range(nkb):
                    j = kb - 4 * qs
                    qoff = max(0, j) * P
                    N = 512 - qoff
                    psT = psB.tile([P, 1024], F32, tag="sT")
                    for h in range(2):
                        nc.tensor.matmul(
                            psT[:, h * 512 + qoff:h * 512 + 512],
                            kT[b][h * HD:(h + 1) * HD,
                                  kb * P:(kb + 1) * P],
                            qT[b][h * HD:(h + 1) * HD,
                                  qs * 512 + qoff:(qs + 1) * 512],
                            start=True, stop=True,
                            tile_position=(h * HD, 0),
                            skip_group_check=True,
                        )
                    pb = ppb.tile([P, 1024], BF16, tag="pb")
                    psT_r = psT[:].rearrange("p (h q) -> p h q", h=2)
                    pb_r = pb[:].rearrange("p (h q) -> p h q", h=2)
                    nc.scalar.activation(pb_r[:, :, qoff:512],
                                         psT_r[:, :, qoff:512],
                                         EXP, scale=0.125)
                    if j >= 0:
                        mr = maskT2_t[:].rearrange("p (h q) -> p h q", h=2)
                        nc.vector.tensor_mul(pb_r[:, :, qoff:qoff + P],
                                             pb_r[:, :, qoff:qoff + P],
                                             mr[:])
                    pend.append((kb, pb, qoff, N))
                    while len(pend) > 2:
                        pv_emit(b, qs, nkb, pctx, pend.pop(0))
                while pend:
                    pv_emit(b, qs, nkb, pctx, pend.pop(0))
                # normalize: recip of sums row (both heads), K=1 broadcasts
                rbf = prb.tile([P, 1024], BF16, tag="rbf")
                with nc.allow_low_precision("softmax denom bcast in bf16"):
                    nc.vector.reciprocal(rbf[64:65, :], pctx[64:65, :])
                pbc = psB.tile([P, 1024], F32, tag="sT")
                nc.tensor.matmul(pbc[0:64, 0:512], onesel_t[64:65, :],
                                 rbf[64:65, 0:512], start=True, stop=True,
                                 skip_group_check=True)
                nc.tensor.matmul(pbc[64:128, 0:512], onesel_t[64:65, :],
                                 rbf[64:65, 512:1024], start=True, stop=True,
                                 skip_group_check=True)
                rbb = prb.tile([P, 512], F32, tag="rbb")
                nc.vector.tensor_copy(rbb[:], pbc[:, 0:512])
                w = slice(qs * 512, (qs + 1) * 512)
                nc.vector.tensor_mul(ctxT[b][0:64, w], pctx[0:64, 0:512],
                                     rbb[0:64, :])
                nc.vector.tensor_mul(ctxT[b][64:128, w],
                                     pctx[0:64, 512:1024],
                                     rbb[64:128, :])
                nc.gpsimd.dma_start(
                    a2a_in[b][2 * qs:2 * qs + 2].rearrange("k p c -> p k c"),
                    ctxT[b][:, w].rearrange("p (k c) -> p k c", k=2))

            def out_proj(bb):
                ctxs = pob.tile([P, 8, 256], BF16, tag="ctxs")
                nc.sync.dma_start(ctxs[:],
                                  a2a_out[bb][:].rearrange("j p c -> p j c"))
                for mt in range(2):
                    ob = pob.tile([P, H], F32, tag="ob")
                    for nt in range(2):
                        po = psB.tile([P, 1024], F32, tag="sT")
                        for jj in range(8):
                            nc.tensor.matmul(
                                po[:, 0:512],
                                ctxs[:, jj, mt * P:(mt + 1) * P],
                                wo[:, jj, nt * 512:(nt + 1) * 512],
                                start=(jj == 0), stop=(jj == 7),
                                skip_group_check=True,
                            )
                        nc.vector.tensor_copy(ob[:, nt * 512:(nt + 1) * 512],
                                              po[:, 0:512])
                    nc.sync.dma_start(
                        out_d[bb * 256 + mt * P:bb * 256 + (mt + 1) * P, :],
                        ob[:])

            def collective(b):
                nc.gpsimd.collective_compute(
                    "AllToAll",
                    mybir.AluOpType.bypass,
                    replica_groups=[list(range(NCORES))],
                    ins=[a2a_in[b].opt()],
                    outs=[a2a_out[b].opt()],
                )

            # ---------------- schedule
            for ch in range(4):
                chunk_qkv(0, ch)
            nc.sync.dma_start(
                wo[:], woutT_d[:].rearrange("(j p) n -> p j n", p=P))
            for qs in (3, 2, 1, 0):
                att_block(0, qs)
                chunk_qkv(1, state["b1ch"])
                state["b1ch"] += 1
            collective(0)
            att_block(1, 3)
            att_block(1, 2)
            out_proj(0)
            att_block(1, 1)
            att_block(1, 0)
            collective(1)
            out_proj(1)

    nc.finalize()
    return nc


_NC_CACHE = None


def _get_nc():
    global _NC_CACHE
    if _NC_CACHE is None:
        _NC_CACHE = _build_nc()
    return _NC_CACHE


# original rope dim -> stored row (per 64-dim head): [0:16, 32:48, 16:32, 48:64]
_ROPE_PERM = np.concatenate([
    np.arange(0, 16), np.arange(32, 48),
    np.arange(16, 32), np.arange(48, 64),
])


def _host_tables():
    import ml_dtypes
    j = np.arange(32)
    inv = (10000.0 ** (-(j.astype(np.float64)) / 32.0))
    pos = np.arange(S, dtype=np.float64)
    fr = pos[:, None] * inv[None, :]              # [S, 32]
    cosT = np.cos(fr).T.astype(np.float32)        # [32, S]
    sinT = np.sin(fr).T.astype(np.float32)
    # stored row r holds original dim d = _ROPE_PERM[r]; freq j = d mod 32,
    # sin sign = -1 for d < 32 (first half), +1 for d >= 32
    cos64 = np.empty((64, S), np.float32)
    sin64 = np.empty((64, S), np.float32)
    for r, d in enumerate(_ROPE_PERM):
        jj = d % 32
        cos64[r] = cosT[jj]
        sin64[r] = sinT[jj] if d >= 32 else -sinT[jj]
    costab = np.tile(cos64, (2, 1))               # [128, S]
    sintab = np.tile(sin64, (2, 1))
    kk = np.arange(P)[:, None]
    qq = np.arange(P)[None, :]
    m = np.where(kk <= qq, 1.0, 0.0).astype(ml_dtypes.bfloat16)
    maskT2 = np.concatenate([m, m], axis=1)       # [128, 256]
    onesel = np.ones((P, 64), dtype=np.float32).astype(ml_dtypes.bfloat16)
    return costab, sintab, maskT2, onesel


def _make_in_maps(x, W_qkv, W_out):
    import ml_dtypes
    costab, sintab, maskT2, onesel = _host_tables()
    xT = np.ascontiguousarray(x.reshape(T, H).T).astype(ml_dtypes.bfloat16)
    woutT = np.ascontiguousarray(W_out.T).astype(ml_dtypes.bfloat16)
    in_maps = []
    for c in range(NCORES):
        h0 = 2 * c
        # q/k weight rows permuted so rope partners share a 32-quadrant
        qrows = W_qkv[HD * h0:HD * (h0 + 2)]
        krows = W_qkv[H + HD * h0:H + HD * (h0 + 2)]
        vrows = W_qkv[2 * H + HD * h0:2 * H + HD * (h0 + 2)]
        perm128 = np.concatenate([_ROPE_PERM, 64 + _ROPE_PERM])
        rows = np.concatenate([qrows[perm128], krows[perm128], vrows], axis=0)
        wqkvT = np.ascontiguousarray(rows.T).astype(ml_dtypes.bfloat16)
        in_maps.append({
            "xT": xT, "wqkvT": wqkvT, "woutT": woutT,
            "costab": costab, "sintab": sintab,
            "maskT2": maskT2, "onesel": onesel,
        })
    return in_maps


def _run_spmd(x, W_qkv, W_out, **kw):
    nc = _get_nc()
    in_maps = _make_in_maps(x, W_qkv, W_out)
    return run_bass_kernel_spmd(nc, in_maps, core_ids=list(range(NCORES)),
                                **kw)


def kernel(x, W_qkv, W_out):
    x = np.asarray(x, dtype=np.float32)
    W_qkv = np.asarray(W_qkv, dtype=np.float32)
    W_out = np.asarray(W_out, dtype=np.float32)
    res = _run_spmd(x, W_qkv, W_out)
    # core c owns tokens [c*256,(c+1)*256) of each batch (flattened b-major)
    full = np.empty((T, H), dtype=np.float32)
    for c in range(NCORES):
        o = res.results[c]["out"]
        full[c * 256:(c + 1) * 256] = o[0:256]
        full[S + c * 256:S + (c + 1) * 256] = o[256:512]
    return full.reshape(B, S, H)
